# revision 1
# baseline (speedup 1.0000x reference)
"""Causal self-attention (B=4, T=2048, C=1024, H=16, D=64) on 8 trn2 cores.

Sharding: core c = 2*b + g handles batch b and head-group g (8 heads each).
Fully communication-free: each core computes the qkv projection for its head
columns, causal attention for its 8 heads, and a partial output projection
(contraction over its 512 head-columns). The host sums the two head-group
partials per batch and adds out_b.

Schedule (single in-order PE stream, other engines trail via Tile deps):
  [qkv block0][attn qt0][qkv block1][attn qt1]...[attn qt3 + out-proj interleaved]
so the Scalar engine (exp: the longest fixed cost) starts ~20us in and stays
saturated to the end, with out-projection matmuls filling the PE during the
exp-bound qt3 tail.

Device notes (per core):
  - scores are computed TRANSPOSED: sT[k, q]; softmax key-sums ride the PE
    via a ones-augmented V (lhsT = [v | 1]); attention output lands as y^T,
    which feeds the output projection lhsT directly (no transposes anywhere).
  - head PAIRS (partitions 0:64 / 64:128) issue score matmuls alternately;
    tile_position auto-derives (0,0)/(64,0) so the two K=64 matmuls run
    concurrently in different PE row groups (2x score throughput).
  - score PSUM tiles are 3-bank [128, 1536] (3 chunk-slots, heads mixed):
    ONE exp per tile. Diagonal chunks compute full width (finite garbage,
    masked after exp); AV matmuls stream only the causally valid columns.
  - normalization is per-iteration: sums row -> sbuf -> K=1 matmul broadcast
    into psum rows 64:128 -> one DVE divide writes normalized y^T.
  - all matmul inputs bf16, accumulation fp32 in PSUM; output staged fp16.
"""
import numpy as np
import ml_dtypes
from contextlib import ExitStack

import concourse.bass as bass
import concourse.mybir as mybir
import concourse.tile as tile
from concourse.masks import make_upper_triangular
from concourse.bass_utils import run_bass_kernel_spmd

BF16 = mybir.dt.bfloat16
F16 = mybir.dt.float16
F32 = mybir.dt.float32

B, T, C = 4, 2048, 1024
H, D = 16, 64
HC = H // 2          # heads per core
P = 128
NQ = 512             # q tile (columns of the transposed score tile)
CK = C // P          # contraction chunks for qkv proj (8)
NT = T // P          # T tiles of 128 (16)
NQT = T // NQ        # q macro tiles / token blocks (4)
HCOL = HC * D        # head columns per core (512)
NHP = HC // 2        # head pairs (4)
SLOTS = 3            # 512-wide chunk slots per score psum tile


def _split_waits(nc):
    """walrus in this container rejects >1 sync wait per instruction; hoist
    extras onto preceding NoOps on the same engine."""
    for func in nc.m.functions:
        for bb in func.blocks:
            newlist = []
            for inst in bb.instructions:
                si = inst.sync_info
                if si is not None and si.on_wait and len(si.on_wait) > 1:
                    extra = list(si.on_wait[:-1])
                    keep = list(si.on_wait[-1:])
                    for j, w in enumerate(extra):
                        newlist.append(mybir.InstNoOp(
                            name=f"{inst.name}-wsplit{j}",
                            sync_info=mybir.SyncInfo(on_wait=[w], on_update=[]),
                            bass_nofuse=True, engine=inst.engine))
                    si.on_wait = keep
                newlist.append(inst)
            bb.instructions = newlist


def _emit(nc, tc, ctx):
    xT_d = nc.dram_tensor("xT", [C, T], BF16, kind="ExternalInput")
    wqk_d = nc.dram_tensor("wqk", [C, 2 * HCOL], BF16, kind="ExternalInput")
    wv_d = nc.dram_tensor("wv", [C, HCOL], BF16, kind="ExternalInput")
    wout_d = nc.dram_tensor("wout", [HCOL, C], BF16, kind="ExternalInput")
    bqk_d = nc.dram_tensor("bqk", [2 * HCOL], F32, kind="ExternalInput")
    bv_d = nc.dram_tensor("bv", [1, HCOL], F32, kind="ExternalInput")
    out_d = nc.dram_tensor("out", [T, C], F16, kind="ExternalOutput")

    consts = ctx.enter_context(tc.tile_pool(name="consts", bufs=1))
    weights = ctx.enter_context(tc.tile_pool(name="weights", bufs=1))
    acts = ctx.enter_context(tc.tile_pool(name="acts", bufs=1))
    pt_pool = ctx.enter_context(tc.tile_pool(name="ptp", bufs=8))
    misc = ctx.enter_context(tc.tile_pool(name="misc", bufs=4))
    outp = ctx.enter_context(tc.tile_pool(name="outp", bufs=3))
    ps_s = ctx.enter_context(tc.tile_pool(name="ps_s", bufs=2, space="PSUM"))
    ps_av = ctx.enter_context(tc.tile_pool(name="ps_av", bufs=2, space="PSUM"))

    # ---- constants ----
    tri01 = consts.tile([P, P], BF16, name="tri01")
    make_upper_triangular(nc, tri01, val=1.0, diag=True)
    ones_row = consts.tile([1, P], F32, name="ones_row")
    nc.vector.memset(ones_row, 1.0)
    bqk_sb = consts.tile([P, 2 * HCOL // P], F32, name="bqk_sb")
    nc.scalar.dma_start(out=bqk_sb, in_=bqk_d.rearrange("(m p) -> p m", p=P))
    bv_row = consts.tile([1, HCOL], F32, name="bv_row")
    nc.scalar.dma_start(out=bv_row, in_=bv_d[:])
    # broadcast v-bias to all 128 partitions via K=1 matmul
    bv_ps = ps_av.tile([P, NQ], F32, name="bv_ps", tag="av")
    nc.tensor.matmul(bv_ps[:, 0:HCOL], lhsT=ones_row, rhs=bv_row,
                     start=True, stop=True)
    bv_full = consts.tile([P, HCOL], F32, name="bv_full")
    nc.vector.tensor_copy(bv_full, bv_ps[:, 0:HCOL])

    # ---- input DMAs, spread across queues ----
    xT_sb = weights.tile([P, CK, T], BF16, name="xT_sb")
    wqk_sb = weights.tile([P, CK, 2 * HCOL], BF16, name="wqk_sb")
    wv_sb = weights.tile([P, CK, HCOL], BF16, name="wv_sb")
    xT_r = xT_d.rearrange("(c p) t -> p c t", p=P)
    wqk_r = wqk_d.rearrange("(c p) n -> p c n", p=P)
    wv_r = wv_d.rearrange("(c p) n -> p c n", p=P)
    for c in range(CK):
        nc.gpsimd.dma_start(out=wqk_sb[:, c], in_=wqk_r[:, c])
    for b in range(NQT):
        nc.sync.dma_start(out=xT_sb[:, :, b * NQ:(b + 1) * NQ],
                          in_=xT_r[:, :, b * NQ:(b + 1) * NQ])
    for c in range(CK):
        nc.gpsimd.dma_start(out=wv_sb[:, c], in_=wv_r[:, c])
    wout_sb = weights.tile([P, HCOL // P, C], BF16, name="wout_sb")
    nc.scalar.dma_start(out=wout_sb,
                        in_=wout_d.rearrange("(c p) n -> p c n", p=P))

    qkT_sb = acts.tile([P, 2 * HCOL // P, T], BF16, name="qkT_sb")
    v_sb = acts.tile([P, NT, HC, D + 1], BF16, name="v_sb")
    yT_sb = acts.tile([P, HCOL // P, T], BF16, name="yT_sb")
    nc.vector.memset(v_sb[:, :, :, D:D + 1], 1.0)
    sums_t = [acts.tile([HC, NQ], F32, name=f"sums{qt}") for qt in range(NQT)]
    rq_t = [acts.tile([HC, NQ], F32, name=f"rq{qt}") for qt in range(NQT)]

    def qk_unit(b, ms):
        ps = ps_s.tile([P, SLOTS * NQ], F32, name="ps_qk", tag="s")
        for si, m in enumerate(ms):
            for c in range(CK):
                nc.tensor.matmul(
                    ps[:, si * NQ:(si + 1) * NQ],
                    lhsT=wqk_sb[:, c, m * P:(m + 1) * P],
                    rhs=xT_sb[:, c, b * NQ:(b + 1) * NQ],
                    start=(c == 0), stop=(c == CK - 1))
        for si, m in enumerate(ms):
            nc.vector.tensor_scalar(
                out=qkT_sb[:, m, b * NQ:(b + 1) * NQ],
                in0=ps[:, si * NQ:(si + 1) * NQ],
                scalar1=bqk_sb[:, m:m + 1], scalar2=None,
                op0=mybir.AluOpType.add)

    def v_unit(ts):
        ps = ps_s.tile([P, SLOTS * NQ], F32, name="ps_v", tag="s")
        for si, t in enumerate(ts):
            for c in range(CK):
                nc.tensor.matmul(
                    ps[:, si * NQ:(si + 1) * NQ],
                    lhsT=xT_sb[:, c, t * P:(t + 1) * P],
                    rhs=wv_sb[:, c, :], start=(c == 0), stop=(c == CK - 1))
        for si, t in enumerate(ts):
            nc.vector.tensor_tensor(
                v_sb[:, t, :, 0:D],
                ps[:, si * NQ:(si + 1) * NQ].rearrange(
                    "p (h d) -> p h d", h=HC),
                bv_full.rearrange("p (h d) -> p h d", h=HC),
                mybir.AluOpType.add)

    def qkv_units(b):
        t0 = b * (NT // NQT)
        return [lambda: qk_unit(b, [0, 1, 2]),
                lambda: qk_unit(b, [3, 4, 5]),
                lambda: qk_unit(b, [6, 7]),
                lambda: v_unit([t0, t0 + 1, t0 + 2]),
                lambda: v_unit([t0 + 3])]

    def norm_units(qt):
        """per-qt batched 8-lane reciprocal; per-head: DMA the reciprocal
        row to partition 0, K=1 matmul broadcast (3 heads per score-pool
        tile), DVE mults. All off the attention critical path."""
        units = [lambda: nc.vector.reciprocal(rq_t[qt], sums_t[qt])]

        def norm_heads(i0):
            ps = ps_s.tile([P, SLOTS * NQ], F32, name="ps_bc", tag="s")
            idx = list(range(i0, min(i0 + SLOTS, HC)))
            for si, i in enumerate(idx):
                rrow = misc.tile([1, NQ], F32, name="rrow", tag="rrow")
                nc.sync.dma_start(out=rrow, in_=rq_t[qt][i:i + 1, :])
                nc.tensor.matmul(
                    ps[0:64, si * NQ:(si + 1) * NQ],
                    lhsT=ones_row[:, 0:64], rhs=rrow,
                    start=True, stop=True)
            for si, i in enumerate(idx):
                ysl = yT_sb[64 * (i % 2):64 * (i % 2) + D, i // 2,
                            qt * NQ:(qt + 1) * NQ]
                nc.vector.tensor_tensor(
                    ysl, ysl, ps[0:64, si * NQ:(si + 1) * NQ],
                    mybir.AluOpType.mult)
        for i0 in range(0, HC, SLOTS):
            units.append(lambda i0=i0: norm_heads(i0))
        return units

    def out_proj_tile(t):
        ot = outp.tile([P, C], F16, name="ot", tag="ot")
        ps = ps_s.tile([P, SLOTS * NQ], F32, name="ps_op", tag="s")
        for half in range(C // NQ):
            for c in range(HCOL // P):
                nc.tensor.matmul(
                    ps[:, half * NQ:(half + 1) * NQ],
                    lhsT=yT_sb[:, c, t * P:(t + 1) * P],
                    rhs=wout_sb[:, c, half * NQ:(half + 1) * NQ],
                    start=(c == 0), stop=(c == HCOL // P - 1))
        nc.vector.tensor_copy(ot, ps[:, 0:C])
        nc.sync.dma_start(out=out_d[t * P:(t + 1) * P, :], in_=ot)

    def outproj_units(qt):
        return [lambda t=t: out_proj_tile(t)
                for t in range(4 * qt, 4 * qt + 4)]

    def attn_qt(qt, fillers):
        """attention for all head pairs at q block qt; fillers (thunks of
        PE work from other phases) are spread evenly between score tiles
        to keep the PE busy while ScalarE chews exp."""
        diag0 = (qt * NQ) // P
        nkc = diag0 + NQ // P
        ntiles = NHP * ((2 * nkc + SLOTS - 1) // SLOTS)
        nfill = len(fillers)
        tcount = popped = 0

        def maybe_fill():
            nonlocal popped, tcount
            tcount += 1
            while fillers and popped < tcount * nfill // ntiles:
                fillers.pop(0)()
                popped += 1

        for hp in range(NHP):
            heads = (2 * hp, 2 * hp + 1)
            pos = [64 * (h % 2) for h in heads]
            qTs = [qkT_sb[pos[e]:pos[e] + D, hp, :] for e in range(2)]
            kTs = [qkT_sb[pos[e]:pos[e] + D, 4 + hp, :] for e in range(2)]
            av = [ps_av.tile([P, NQ], F32, name=f"av{e}", tag="av")
                  for e in range(2)]
            # interleaved slot list: (e, kc) alternating heads
            slots = [(e, kc) for kc in range(nkc) for e in range(2)]
            tiles = [slots[i:i + SLOTS] for i in range(0, len(slots), SLOTS)]
            filled = []    # (ps, pt, tile_slots) fills awaiting exp/mask/AV

            def emit_av(ps, pt, tslots):
                nc.scalar.activation(
                    pt[:, 0:len(tslots) * NQ], ps[:, 0:len(tslots) * NQ],
                    mybir.ActivationFunctionType.Exp, scale=float(D) ** -0.5)
                for si, (e, kc) in enumerate(tslots):
                    r = kc - diag0
                    if r >= 0:
                        qoff = r * P
                        nc.vector.tensor_tensor(
                            pt[:, si * NQ + qoff:si * NQ + qoff + P],
                            pt[:, si * NQ + qoff:si * NQ + qoff + P],
                            tri01, mybir.AluOpType.mult)
                for si, (e, kc) in enumerate(tslots):
                    qoff = max(0, kc - diag0) * P
                    nc.tensor.matmul(
                        av[e][0:D + 1, qoff:NQ],
                        lhsT=v_sb[:, kc, heads[e], :],
                        rhs=pt[:, si * NQ + qoff:(si + 1) * NQ],
                        start=(kc == 0), stop=(kc == nkc - 1))

            for tslots in tiles:
                ps = ps_s.tile([P, SLOTS * NQ], F32, name="ps_sc", tag="s")
                pt = pt_pool.tile([P, SLOTS * NQ], BF16, name="pt", tag="pt")
                for si, (e, kc) in enumerate(tslots):
                    nc.tensor.matmul(
                        ps[:, si * NQ:(si + 1) * NQ],
                        lhsT=kTs[e][:, kc * P:(kc + 1) * P],
                        rhs=qTs[e][:, qt * NQ:(qt + 1) * NQ],
                        start=True, stop=True)
                filled.append((ps, pt, tslots))
                if len(filled) > 1:
                    emit_av(*filled.pop(0))
                    maybe_fill()
            for f in filled:
                emit_av(*f)
                maybe_fill()
            # stage the sums row (DMA: PSUM row 64 -> sums partition 2hp+e)
            # and the unnormalized y^T; reciprocal + normalization run
            # batched per qt (norm_units) later
            for e in range(2):
                srow = misc.tile([1, NQ], F32, name="srow", tag="srow")
                nc.vector.tensor_copy(srow, av[e][D:D + 1, :])
                nc.gpsimd.dma_start(
                    out=sums_t[qt][2 * hp + e:2 * hp + e + 1, :],
                    in_=srow)
                nc.vector.tensor_copy(
                    yT_sb[pos[e]:pos[e] + D, hp, qt * NQ:(qt + 1) * NQ],
                    av[e][0:D, :])
        while fillers:
            fillers.pop(0)()

    # ---- main schedule: attention backbone with PE filler injection ----
    for u in qkv_units(0):
        u()
    attn_qt(0, qkv_units(1))
    attn_qt(1, norm_units(0) + qkv_units(2))
    attn_qt(2, norm_units(1) + outproj_units(0) + qkv_units(3))
    attn_qt(3, norm_units(2) + outproj_units(1) + outproj_units(2))
    for u in norm_units(3) + outproj_units(3):
        u()


_NC = None


def _build():
    global _NC
    if _NC is None:
        nc = bass.Bass("TRN2")
        with tile.TileContext(nc) as tc, ExitStack() as ctx:
            _emit(nc, tc, ctx)
        _split_waits(nc)
        _NC = nc
    return _NC


def _in_maps(x, qkv_w, qkv_b, out_w):
    x = np.asarray(x, np.float32)
    qkv_w = np.asarray(qkv_w, np.float32)
    qkv_b = np.asarray(qkv_b, np.float32)
    out_w = np.asarray(out_w, np.float32)
    maps = []
    xTs = [np.ascontiguousarray(x[b].T).astype(ml_dtypes.bfloat16)
           for b in range(B)]
    for core in range(2 * B):
        b, g = core // 2, core % 2
        lo = g * HCOL
        wq = qkv_w[:, lo:lo + HCOL]
        wk = qkv_w[:, C + lo:C + lo + HCOL]
        wv = qkv_w[:, 2 * C + lo:2 * C + lo + HCOL]
        bq = qkv_b[lo:lo + HCOL]
        bk = qkv_b[C + lo:C + lo + HCOL]
        bv = qkv_b[2 * C + lo:2 * C + lo + HCOL]
        wout = out_w[lo:lo + HCOL, :]
        maps.append({
            "xT": xTs[b],
            "wqk": np.concatenate([wq, wk], 1).astype(ml_dtypes.bfloat16),
            "wv": wv.astype(ml_dtypes.bfloat16),
            "wout": np.ascontiguousarray(wout).astype(ml_dtypes.bfloat16),
            "bqk": np.concatenate([bq, bk]).astype(np.float32),
            "bv": bv[None, :].astype(np.float32),
        })
    return maps


def run(x, qkv_w, qkv_b, out_w, out_b, trace=False, tmpdir=None):
    nc = _build()
    maps = _in_maps(x, qkv_w, qkv_b, out_w)
    res = run_bass_kernel_spmd(nc, maps, core_ids=list(range(2 * B)),
                               trace=trace, tmpdir=tmpdir)
    out_b = np.asarray(out_b, np.float32)
    out = np.empty((B, T, C), np.float32)
    for b in range(B):
        out[b] = np.asarray(res.results[2 * b]["out"], np.float32) \
            + np.asarray(res.results[2 * b + 1]["out"], np.float32) \
            + out_b[None, :]
    return out, res


def kernel(x, qkv_w, qkv_b, out_w, out_b):
    out, _ = run(x, qkv_w, qkv_b, out_w, out_b, trace=False)
    return out

